# revision 5
# baseline (speedup 1.0000x reference)
"""Trainium2 Bass kernel for conv_downsample_3d (StyleGAN2-style):
separable 5-tap FIR smoothing ([1,3,3,3,1]/11 per axis) followed by a
3x3x3 stride-2 conv (128->256 ch) plus bias.

x: (2,128,48,48,48) f32, weight: (256,128,3,3,3), bias: (256,)
out: (2,256,24,24,24) f32

Strategy (8 NeuronCores, SPMD):
 - shard over (batch n in 2) x (output-depth quarter dq in 4); each core
   computes out[n, :, 6*dq:6*dq+6, :, :] (full 256 out-channels).
 - all matmuls in float32r (fp32 with 12-bit-rounded mantissa, full PE rate):
   * FIR pass per axis = 5 PSUM-accumulated matmuls with k_t*I stationary
     and shifted rhs access patterns (shift lives in the free-dim AP).
     Tap coefficients {1,3} and 12x12-bit products are exact; the 1/11^3
     normalization is folded into the conv weights on the host.
   * conv = 27 tap-matmuls (lhsT = W_tap[ci,oc]) accumulating into PSUM,
     strided rhs APs over the FIR-ed planes; bias fused into the ACT
     eviction (out = psum + bias per partition).
"""

import os

import numpy as np

_CACHE = {}

N_CORES = 8
FIR_G = [0, 1, 1, 1, 0]  # identity block per tap: 0 -> 1*I, 1 -> 3*I

# fp32r matmuls stream column pairs: every moving-operand size must be EVEN
# (and >=256 for full rate). Planes are padded to width 54 so chunks stay even.
D_CHUNKS = [(0, 512), (512, 512), (1024, 512), (1536, 512), (2048, 512), (2560, 302)]
H_CHUNKS_49 = [(0, 9), (9, 8), (17, 8), (25, 8), (33, 8), (41, 8)]
W_CHUNKS_49 = [(0, 10), (10, 10), (20, 10), (30, 10), (40, 9)]


def _round_fp32r(a: np.ndarray) -> np.ndarray:
    """Round-to-nearest-even onto the fp32r grid (low 12 mantissa bits)."""
    a = np.ascontiguousarray(a, dtype=np.float32)
    u = a.view(np.uint32).astype(np.uint64)
    u = ((u + 0x7FF + ((u >> 12) & 1)) & ~np.uint64(0xFFF)).astype(np.uint32)
    return u.view(np.float32)


def _maybe_install_ntff_shim():
    """Best-effort: register the axon NTFF profiling hook so BASS_TRACE=1
    yields exec times. Harmless if unavailable."""
    try:
        import sys
        import types

        if "antenv.axon_hooks" not in sys.modules:
            mod = types.ModuleType("antenv.axon_hooks")
            holder = {"hook": None}
            mod.set_axon_ntff_profile_hook = lambda h: holder.__setitem__("hook", h)
            mod.get_axon_ntff_profile_hook = lambda: holder["hook"]
            sys.modules["antenv.axon_hooks"] = mod
        mod = sys.modules["antenv.axon_hooks"]
        if mod.get_axon_ntff_profile_hook() is None:
            from trn_agent_boot.trn_boot import _ntff_profile_via_ctypes

            mod.set_axon_ntff_profile_hook(
                _ntff_profile_via_ctypes("/opt/axon/libaxon_pjrt.so")
            )
    except Exception:
        pass


def _build_module():
    import concourse.bacc as bacc
    import concourse.mybir as mybir
    import concourse.tile as tile

    dt = mybir.dt
    nc = bacc.Bacc("TRN2", target_bir_lowering=False, debug=False)

    xs = nc.dram_tensor("xs", [128, 17, 53, 54], dt.float32r, kind="ExternalInput").ap()
    wt = nc.dram_tensor("wt", [128, 27 * 256], dt.float32r, kind="ExternalInput").ap()
    ident = nc.dram_tensor("ident", [128, 256], dt.float32r, kind="ExternalInput").ap()
    biasd = nc.dram_tensor("biasd", [128, 2], dt.float32, kind="ExternalInput").ap()
    o = nc.dram_tensor("o", [256, 6, 24, 24], dt.float32, kind="ExternalOutput").ap()

    with tile.TileContext(nc) as tc:
        with (
            tc.tile_pool(name="wp", bufs=1) as wpool,
            tc.tile_pool(name="raw", bufs=5) as rawpool,
            tc.tile_pool(name="midd", bufs=2) as midpool,
            tc.tile_pool(name="xfp", bufs=4) as xfpool,
            tc.tile_pool(name="stp", bufs=3) as stpool,
            tc.tile_pool(name="fps", bufs=4, space="PSUM") as fps,
            tc.tile_pool(name="cps", bufs=4, space="PSUM") as cps,
        ):
            wtile = wpool.tile([128, 27 * 256], dt.float32r, tag="wt", name="wtile")
            nc.sync.dma_start(wtile[:], wt[:])
            itile = wpool.tile([128, 256], dt.float32r, tag="ident", name="itile")
            nc.sync.dma_start(itile[:], ident[:])
            btile = wpool.tile([128, 2], dt.float32, tag="bias", name="btile")
            nc.sync.dma_start(btile[:], biasd[:])

            raw = {}

            def load_raw(p):
                t = rawpool.tile([128, 53, 54], dt.float32r, tag="raw", name=f"raw{p}")
                nc.sync.dma_start(t[:], xs[:, p, :, :])
                raw[p] = t

            for p in range(4):
                load_raw(p)

            xf = {}
            for j in range(13):
                load_raw(j + 4)

                # ---- d-pass (elementwise across planes): flat even chunks
                xd = midpool.tile([128, 53, 54], dt.float32r, tag="xd", name=f"xd{j}")
                xdf = xd[:].rearrange("p a b -> p (a b)")
                for ci, (c0, nc_) in enumerate(D_CHUNKS):
                    ps = fps.tile([128, 512], dt.float32, tag="fir", name=f"dp{j}_{ci}")
                    for t in range(5):
                        g = FIR_G[t]
                        nc.tensor.matmul(
                            ps[:, :nc_],
                            itile[:, g * 128 : (g + 1) * 128],
                            raw[j + t][:].rearrange("p a b -> p (a b)")[:, c0 : c0 + nc_],
                            start=(t == 0),
                            stop=(t == 4),
                        )
                    nc.vector.tensor_copy(xdf[:, c0 : c0 + nc_], ps[:, :nc_])

                # ---- h-pass: x1[h, wp] = sum_t k_t xd[h+t, wp]
                x1 = midpool.tile([128, 49, 54], dt.float32r, tag="x1", name=f"x1_{j}")
                for ci, (r0, nr) in enumerate(H_CHUNKS_49):
                    ps = fps.tile([128, nr, 54], dt.float32, tag="fir", name=f"hp{j}_{ci}")
                    for t in range(5):
                        g = FIR_G[t]
                        nc.tensor.matmul(
                            ps[:],
                            itile[:, g * 128 : (g + 1) * 128],
                            xd[:, r0 + t : r0 + t + nr, :],
                            start=(t == 0),
                            stop=(t == 4),
                        )
                    nc.vector.tensor_copy(x1[:, r0 : r0 + nr, :], ps[:])

                # ---- w-pass: xf[h, w] = sum_t k_t x1[h, w+t]  (w padded to 50)
                xfj = xfpool.tile([128, 49, 50], dt.float32r, tag="xf", name=f"xf{j}")
                for ci, (r0, nr) in enumerate(W_CHUNKS_49):
                    ps = fps.tile([128, nr, 50], dt.float32, tag="fir", name=f"wp{j}_{ci}")
                    for t in range(5):
                        g = FIR_G[t]
                        nc.tensor.matmul(
                            ps[:],
                            itile[:, g * 128 : (g + 1) * 128],
                            x1[:, r0 : r0 + nr, t : t + 50],
                            start=(t == 0),
                            stop=(t == 4),
                        )
                    nc.vector.tensor_copy(xfj[:, r0 : r0 + nr, :], ps[:])
                xf[j] = xfj

                # ---- conv for output plane sd once xf[2sd..2sd+2] ready
                if j >= 2 and j % 2 == 0:
                    sd = (j - 2) // 2
                    for b in range(2):
                        stage = stpool.tile(
                            [128, 24, 24], dt.float32, tag="st", name=f"st{sd}_{b}"
                        )
                        for hh in range(2):
                            pc = cps.tile(
                                [128, 12, 24], dt.float32, tag="conv", name=f"cv{sd}_{b}_{hh}"
                            )
                            tapi = 0
                            for kd in range(3):
                                src = xf[2 * sd + kd]
                                for kh in range(3):
                                    h0 = 24 * hh + kh
                                    for kw in range(3):
                                        nc.tensor.matmul(
                                            pc[:],
                                            wtile[:, tapi * 256 + b * 128 : tapi * 256 + (b + 1) * 128],
                                            src[:, h0 : h0 + 23 : 2, kw : kw + 47 : 2],
                                            start=(tapi == 0),
                                            stop=(tapi == 26),
                                        )
                                        tapi += 1
                            nc.scalar.activation(
                                stage[:, hh * 12 : (hh + 1) * 12, :],
                                pc[:],
                                mybir.ActivationFunctionType.Identity,
                                bias=btile[:, b : b + 1],
                            )
                        nc.sync.dma_start(o[b * 128 : (b + 1) * 128, sd, :, :], stage[:])

    nc.compile()
    return nc


def _prep_host_inputs(x, weight, bias):
    """Build per-core input maps."""
    x = np.ascontiguousarray(x, dtype=np.float32)
    w64 = np.asarray(weight, dtype=np.float64) / (11.0**3)
    # wt[ci, tap*256 + b*128 + oc] = w64[b*128+oc, ci, kd, kh, kw]
    wt = np.transpose(w64, (1, 2, 3, 4, 0)).reshape(128, 27 * 256)
    wt_np = _round_fp32r(wt.astype(np.float32))

    ident = np.zeros((128, 256), dtype=np.float32)
    ident[:, 0:128] = np.eye(128, dtype=np.float32)
    ident[:, 128:256] = 3.0 * np.eye(128, dtype=np.float32)

    biasb = np.ascontiguousarray(
        np.asarray(bias, dtype=np.float32).reshape(2, 128).T
    )  # [oc, b]

    # pad d,h by (3,2); pad w by (3,3) so plane width 54 keeps chunks even
    xp = np.pad(x, ((0, 0), (0, 0), (3, 2), (3, 2), (3, 3)))
    xp = _round_fp32r(xp)

    in_maps = []
    for core in range(N_CORES):
        n, dq = core // 4, core % 4
        slab = np.ascontiguousarray(xp[n, :, 12 * dq : 12 * dq + 17, :, :])
        in_maps.append(
            {"xs": slab, "wt": wt_np, "ident": ident, "biasd": biasb}
        )
    return in_maps


LAST_RESULTS = None


def kernel(x, weight, bias):
    global LAST_RESULTS
    _maybe_install_ntff_shim()

    from concourse.bass_utils import run_bass_kernel_spmd

    nc = _CACHE.get("nc")
    if nc is None:
        nc = _build_module()
        _CACHE["nc"] = nc

    in_maps = _prep_host_inputs(x, weight, bias)
    res = run_bass_kernel_spmd(nc, in_maps, core_ids=list(range(N_CORES)))
    LAST_RESULTS = res

    out = np.empty((2, 256, 24, 24, 24), dtype=np.float32)
    for core in range(N_CORES):
        n, dq = core // 4, core % 4
        out[n, :, 6 * dq : 6 * dq + 6, :, :] = res.results[core]["o"]
    return out


# revision 7
# speedup vs baseline: 1.2862x; 1.2862x over previous
"""Trainium2 Bass kernel for conv_downsample_3d (StyleGAN2-style):
separable 5-tap FIR smoothing ([1,3,3,3,1]/11 per axis) followed by a
3x3x3 stride-2 conv (128->256 ch) plus bias.

x: (2,128,48,48,48) f32, weight: (256,128,3,3,3), bias: (256,)
out: (2,256,24,24,24) f32

Strategy (8 NeuronCores, SPMD):
 - shard over (batch n in 2) x (output-depth quarter dq in 4); each core
   computes out[n, :, 6*dq:6*dq+6, :, :] (full 256 out-channels).
 - all matmuls in float32r (fp32 with 12-bit-rounded mantissa, full PE rate;
   moving sizes must be even and >=256, inner dims contiguous for pair rate):
   * FIR pass per axis = 5 PSUM-accumulated matmuls with k_t*I stationary
     and shifted rhs access patterns. Tap coefficients {1,3} and 12x12-bit
     products are exact; 1/11^3 normalization is folded into conv weights.
     d- and h- passes compute only the interior (borders stay exactly zero).
   * FIR output planes are stored split into even/odd w-phases so the
     stride-2 conv rhs reads contiguous spans (strided inner = half rate).
   * conv = 27 tap-matmuls accumulating into PSUM; bias fused into the
     ACT eviction.
"""

import numpy as np

_CACHE = {}

N_CORES = 8
FIR_G = [0, 1, 1, 1, 0]  # identity block per tap: 0 -> 1*I, 1 -> 3*I

# interior-only chunks: d-pass over 48 interior rows, inner 48
D_CHUNKS = [(0, 8), (8, 8), (16, 8), (24, 8), (32, 8), (40, 8)]  # rows of 48, N=384
H_CHUNKS_49 = [(0, 9), (9, 8), (17, 8), (25, 8), (33, 8), (41, 8)]  # rows of 49, N=432/384
W_CHUNKS_49 = [(0, 10), (10, 10), (20, 10), (30, 10), (40, 9)]  # rows of 49, N=500/450


def _round_fp32r(a: np.ndarray) -> np.ndarray:
    """Round-to-nearest-even onto the fp32r grid (low 12 mantissa bits)."""
    a = np.ascontiguousarray(a, dtype=np.float32)
    u = a.view(np.uint32).astype(np.uint64)
    u = ((u + 0x7FF + ((u >> 12) & 1)) & ~np.uint64(0xFFF)).astype(np.uint32)
    return u.view(np.float32)


def _maybe_install_ntff_shim():
    """Best-effort: register the axon NTFF profiling hook so BASS_TRACE=1
    yields exec times. Harmless if unavailable."""
    try:
        import sys
        import types

        if "antenv.axon_hooks" not in sys.modules:
            mod = types.ModuleType("antenv.axon_hooks")
            holder = {"hook": None}
            mod.set_axon_ntff_profile_hook = lambda h: holder.__setitem__("hook", h)
            mod.get_axon_ntff_profile_hook = lambda: holder["hook"]
            sys.modules["antenv.axon_hooks"] = mod
        mod = sys.modules["antenv.axon_hooks"]
        if mod.get_axon_ntff_profile_hook() is None:
            from trn_agent_boot.trn_boot import _ntff_profile_via_ctypes

            mod.set_axon_ntff_profile_hook(
                _ntff_profile_via_ctypes("/opt/axon/libaxon_pjrt.so")
            )
    except Exception:
        pass


def _build_module():
    import concourse.bacc as bacc
    import concourse.mybir as mybir
    import concourse.tile as tile

    dt = mybir.dt
    nc = bacc.Bacc("TRN2", target_bir_lowering=False, debug=False)

    xs = nc.dram_tensor("xs", [128, 17, 53, 54], dt.float32r, kind="ExternalInput").ap()
    wt = nc.dram_tensor("wt", [128, 27 * 256], dt.float32r, kind="ExternalInput").ap()
    ident = nc.dram_tensor("ident", [128, 256], dt.float32r, kind="ExternalInput").ap()
    biasd = nc.dram_tensor("biasd", [128, 2], dt.float32, kind="ExternalInput").ap()
    o = nc.dram_tensor("o", [256, 6, 24, 24], dt.float32, kind="ExternalOutput").ap()

    with tile.TileContext(nc) as tc:
        with (
            tc.tile_pool(name="wp", bufs=1) as wpool,
            tc.tile_pool(name="raw", bufs=5) as rawpool,
            tc.tile_pool(name="midd", bufs=1) as midpool,
            tc.tile_pool(name="xfp", bufs=4) as xfpool,
            tc.tile_pool(name="stp", bufs=3) as stpool,
            tc.tile_pool(name="fps", bufs=4, space="PSUM") as fps,
            tc.tile_pool(name="cps", bufs=4, space="PSUM") as cps,
        ):
            # identity first: needed by the very first matmul
            itile = wpool.tile([128, 256], dt.float32r, tag="ident", name="itile")
            nc.sync.dma_start(itile[:], ident[:])

            raw = {}

            def load_raw(p):
                t = rawpool.tile([128, 53, 54], dt.float32r, tag="raw", name=f"raw{p}")
                nc.sync.dma_start(t[:], xs[:, p, :, :])
                raw[p] = t

            for p in range(5):
                load_raw(p)

            # weights/bias: not needed until the first conv (j=2)
            wtile = wpool.tile([128, 27 * 256], dt.float32r, tag="wt", name="wtile")
            nc.sync.dma_start(wtile[:], wt[:])
            btile = wpool.tile([128, 2], dt.float32, tag="bias", name="btile")
            nc.sync.dma_start(btile[:], biasd[:])

            # persistent double-buffered intermediates with exactly-zero borders
            xd2 = [
                midpool.tile([128, 53, 54], dt.float32r, tag=f"xd{i}", name=f"xd{i}")
                for i in range(2)
            ]
            x12 = [
                midpool.tile([128, 49, 54], dt.float32r, tag=f"x1_{i}", name=f"x1_{i}")
                for i in range(2)
            ]
            for i in range(2):
                nc.gpsimd.memset(xd2[i][:].bitcast(dt.float32), 0.0)
                nc.gpsimd.memset(x12[i][:].bitcast(dt.float32), 0.0)

            xf = {}
            for j in range(13):
                if j + 5 <= 16:
                    load_raw(j + 5)

                # ---- d-pass (interior only): xd[3+r, 3+c] over 48x48
                xd = xd2[j % 2]
                for ci, (r0, nr) in enumerate(D_CHUNKS):
                    ps = fps.tile([128, nr, 48], dt.float32, tag="fir", name=f"dp{j}_{ci}")
                    for t in range(5):
                        g = FIR_G[t]
                        nc.tensor.matmul(
                            ps[:],
                            itile[:, g * 128 : (g + 1) * 128],
                            raw[j + t][:, 3 + r0 : 3 + r0 + nr, 3:51],
                            start=(t == 0),
                            stop=(t == 4),
                        )
                    nc.vector.tensor_copy(xd[:, 3 + r0 : 3 + r0 + nr, 3:51], ps[:])

                # ---- h-pass (interior cols): x1[h, 3+c] = sum_t k_t xd[h+t, 3+c]
                x1 = x12[j % 2]
                for ci, (r0, nr) in enumerate(H_CHUNKS_49):
                    ps = fps.tile([128, nr, 48], dt.float32, tag="fir", name=f"hp{j}_{ci}")
                    for t in range(5):
                        g = FIR_G[t]
                        nc.tensor.matmul(
                            ps[:],
                            itile[:, g * 128 : (g + 1) * 128],
                            xd[:, r0 + t : r0 + t + nr, 3:51],
                            start=(t == 0),
                            stop=(t == 4),
                        )
                    nc.vector.tensor_copy(x1[:, r0 : r0 + nr, 3:51], ps[:])

                # ---- w-pass: xf[h, w] = sum_t k_t x1[h, w+t], stored as
                # even/odd w-phases: xfj[:, ph, h, u] = xf[h, 2u+ph]
                xfj = xfpool.tile([128, 2, 49, 25], dt.float32r, tag="xf", name=f"xf{j}")
                for ci, (r0, nr) in enumerate(W_CHUNKS_49):
                    ps = fps.tile([128, nr, 50], dt.float32, tag="fir", name=f"wp{j}_{ci}")
                    for t in range(5):
                        g = FIR_G[t]
                        nc.tensor.matmul(
                            ps[:],
                            itile[:, g * 128 : (g + 1) * 128],
                            x1[:, r0 : r0 + nr, t : t + 50],
                            start=(t == 0),
                            stop=(t == 4),
                        )
                    nc.vector.tensor_copy(
                        xfj[:, 0, r0 : r0 + nr, :], ps[:, :, 0:49:2]
                    )
                    nc.vector.tensor_copy(
                        xfj[:, 1, r0 : r0 + nr, :], ps[:, :, 1:50:2]
                    )
                xf[j] = xfj

                # ---- conv for output plane sd once xf[2sd..2sd+2] ready
                if j >= 2 and j % 2 == 0:
                    sd = (j - 2) // 2
                    for b in range(2):
                        stage = stpool.tile(
                            [128, 24, 24], dt.float32, tag="st", name=f"st{sd}_{b}"
                        )
                        for hh in range(2):
                            pc = cps.tile(
                                [128, 12, 24], dt.float32, tag="conv", name=f"cv{sd}_{b}_{hh}"
                            )
                            tapi = 0
                            for kd in range(3):
                                src = xf[2 * sd + kd]
                                for kh in range(3):
                                    h0 = 24 * hh + kh
                                    for kw in range(3):
                                        ph, off = kw & 1, kw >> 1
                                        nc.tensor.matmul(
                                            pc[:],
                                            wtile[:, tapi * 256 + b * 128 : tapi * 256 + (b + 1) * 128],
                                            src[:, ph, h0 : h0 + 23 : 2, off : off + 24],
                                            start=(tapi == 0),
                                            stop=(tapi == 26),
                                        )
                                        tapi += 1
                            nc.scalar.activation(
                                stage[:, hh * 12 : (hh + 1) * 12, :],
                                pc[:],
                                mybir.ActivationFunctionType.Identity,
                                bias=btile[:, b : b + 1],
                            )
                        nc.sync.dma_start(o[b * 128 : (b + 1) * 128, sd, :, :], stage[:])

    nc.compile()
    return nc


def _prep_host_inputs(x, weight, bias):
    """Build per-core input maps."""
    x = np.ascontiguousarray(x, dtype=np.float32)
    w64 = np.asarray(weight, dtype=np.float64) / (11.0**3)
    # wt[ci, tap*256 + b*128 + oc] = w64[b*128+oc, ci, kd, kh, kw]
    wt = np.transpose(w64, (1, 2, 3, 4, 0)).reshape(128, 27 * 256)
    wt_np = _round_fp32r(wt.astype(np.float32))

    ident = np.zeros((128, 256), dtype=np.float32)
    ident[:, 0:128] = np.eye(128, dtype=np.float32)
    ident[:, 128:256] = 3.0 * np.eye(128, dtype=np.float32)

    biasb = np.ascontiguousarray(
        np.asarray(bias, dtype=np.float32).reshape(2, 128).T
    )  # [oc, b]

    # pad d,h by (3,2); pad w by (3,3) so plane width 54 keeps chunks even
    xp = np.pad(x, ((0, 0), (0, 0), (3, 2), (3, 2), (3, 3)))
    xp = _round_fp32r(xp)

    in_maps = []
    for core in range(N_CORES):
        n, dq = core // 4, core % 4
        slab = np.ascontiguousarray(xp[n, :, 12 * dq : 12 * dq + 17, :, :])
        in_maps.append({"xs": slab, "wt": wt_np, "ident": ident, "biasd": biasb})
    return in_maps


LAST_RESULTS = None


def kernel(x, weight, bias):
    global LAST_RESULTS
    _maybe_install_ntff_shim()

    from concourse.bass_utils import run_bass_kernel_spmd

    nc = _CACHE.get("nc")
    if nc is None:
        nc = _build_module()
        _CACHE["nc"] = nc

    in_maps = _prep_host_inputs(x, weight, bias)
    res = run_bass_kernel_spmd(nc, in_maps, core_ids=list(range(N_CORES)))
    LAST_RESULTS = res

    out = np.empty((2, 256, 24, 24, 24), dtype=np.float32)
    for core in range(N_CORES):
        n, dq = core // 4, core % 4
        out[n, :, 6 * dq : 6 * dq + 6, :, :] = res.results[core]["o"]
    return out


# revision 8
# speedup vs baseline: 1.4525x; 1.1293x over previous
"""Trainium2 Bass kernel for conv_downsample_3d (StyleGAN2-style):
separable 5-tap FIR smoothing ([1,3,3,3,1]/11 per axis) followed by a
3x3x3 stride-2 conv (128->256 ch) plus bias.

x: (2,128,48,48,48) f32, weight: (256,128,3,3,3), bias: (256,)
out: (2,256,24,24,24) f32

Strategy (8 NeuronCores, SPMD):
 - shard over (batch n in 2) x (output-depth quarter dq in 4); each core
   computes out[n, :, 6*dq:6*dq+6, :, :] (full 256 out-channels).
 - all matmuls in float32r (fp32 with 12-bit-rounded mantissa, full PE rate;
   moving sizes must be even and >=256, inner dims contiguous for pair rate):
   * FIR pass per axis = 5 PSUM-accumulated matmuls with k_t*I stationary
     and shifted rhs access patterns. Tap coefficients {1,3} and 12x12-bit
     products are exact; 1/11^3 normalization is folded into conv weights.
     d- and h- passes compute only the interior (borders stay exactly zero).
   * FIR output planes are stored split into even/odd w-phases so the
     stride-2 conv rhs reads contiguous spans (strided inner = half rate).
   * conv = 27 tap-matmuls accumulating into PSUM; bias fused into the
     ACT eviction.
"""

import numpy as np

_CACHE = {}

N_CORES = 8
FIR_G = [0, 1, 1, 1, 0]  # identity block per tap: 0 -> 1*I, 1 -> 3*I

# interior-only chunks: d-pass over 48 interior rows, inner 48
D_CHUNKS = [(0, 10), (10, 10), (20, 10), (30, 10), (40, 8)]  # rows of 48, N=480/384
H_CHUNKS_49 = [(0, 10), (10, 10), (20, 10), (30, 10), (40, 9)]  # rows of 49, N=480/432
W_CHUNKS_49 = [(0, 10), (10, 10), (20, 10), (30, 10), (40, 9)]  # rows of 49, N=500/450
PE_TAPS = [0, 1, 3, 4]  # center tap (t=2, coeff 3) fused into the DVE eviction


def _round_fp32r(a: np.ndarray) -> np.ndarray:
    """Round-to-nearest-even onto the fp32r grid (low 12 mantissa bits)."""
    a = np.ascontiguousarray(a, dtype=np.float32)
    u = a.view(np.uint32).astype(np.uint64)
    u = ((u + 0x7FF + ((u >> 12) & 1)) & ~np.uint64(0xFFF)).astype(np.uint32)
    return u.view(np.float32)


def _maybe_install_ntff_shim():
    """Best-effort: register the axon NTFF profiling hook so BASS_TRACE=1
    yields exec times. Harmless if unavailable."""
    try:
        import sys
        import types

        if "antenv.axon_hooks" not in sys.modules:
            mod = types.ModuleType("antenv.axon_hooks")
            holder = {"hook": None}
            mod.set_axon_ntff_profile_hook = lambda h: holder.__setitem__("hook", h)
            mod.get_axon_ntff_profile_hook = lambda: holder["hook"]
            sys.modules["antenv.axon_hooks"] = mod
        mod = sys.modules["antenv.axon_hooks"]
        if mod.get_axon_ntff_profile_hook() is None:
            from trn_agent_boot.trn_boot import _ntff_profile_via_ctypes

            mod.set_axon_ntff_profile_hook(
                _ntff_profile_via_ctypes("/opt/axon/libaxon_pjrt.so")
            )
    except Exception:
        pass


def _build_module():
    import concourse.bacc as bacc
    import concourse.mybir as mybir
    import concourse.tile as tile

    dt = mybir.dt
    nc = bacc.Bacc("TRN2", target_bir_lowering=False, debug=False)

    xs = nc.dram_tensor("xs", [128, 17, 53, 54], dt.float32r, kind="ExternalInput").ap()
    wt = nc.dram_tensor("wt", [128, 27 * 256], dt.float32r, kind="ExternalInput").ap()
    ident = nc.dram_tensor("ident", [128, 256], dt.float32r, kind="ExternalInput").ap()
    biasd = nc.dram_tensor("biasd", [128, 2], dt.float32, kind="ExternalInput").ap()
    o = nc.dram_tensor("o", [256, 6, 24, 24], dt.float32, kind="ExternalOutput").ap()

    with tile.TileContext(nc) as tc:
        with (
            tc.tile_pool(name="wp", bufs=1) as wpool,
            tc.tile_pool(name="raw", bufs=5) as rawpool,
            tc.tile_pool(name="midd", bufs=1) as midpool,
            tc.tile_pool(name="xfp", bufs=4) as xfpool,
            tc.tile_pool(name="stp", bufs=3) as stpool,
            tc.tile_pool(name="fps", bufs=4, space="PSUM") as fps,
            tc.tile_pool(name="cps", bufs=4, space="PSUM") as cps,
        ):
            # identity first: needed by the very first matmul
            itile = wpool.tile([128, 256], dt.float32r, tag="ident", name="itile")
            nc.sync.dma_start(itile[:], ident[:])

            raw = {}

            def load_raw(p):
                t = rawpool.tile([128, 53, 54], dt.float32r, tag="raw", name=f"raw{p}")
                nc.sync.dma_start(t[:], xs[:, p, :, :])
                raw[p] = t

            for p in range(5):
                load_raw(p)

            # weights/bias: not needed until the first conv (j=2)
            wtile = wpool.tile([128, 27 * 256], dt.float32r, tag="wt", name="wtile")
            nc.sync.dma_start(wtile[:], wt[:])
            btile = wpool.tile([128, 2], dt.float32, tag="bias", name="btile")
            nc.sync.dma_start(btile[:], biasd[:])

            # persistent double-buffered intermediates with exactly-zero borders
            xd2 = [
                midpool.tile([128, 53, 54], dt.float32r, tag=f"xd{i}", name=f"xd{i}")
                for i in range(2)
            ]
            x12 = [
                midpool.tile([128, 49, 54], dt.float32r, tag=f"x1_{i}", name=f"x1_{i}")
                for i in range(2)
            ]
            for i in range(2):
                nc.gpsimd.memset(xd2[i][:].bitcast(dt.float32), 0.0)
                nc.gpsimd.memset(x12[i][:].bitcast(dt.float32), 0.0)

            xf = {}
            for j in range(13):
                if j + 5 <= 16:
                    load_raw(j + 5)

                # ---- d-pass (interior only): xd[3+r, 3+c] over 48x48
                xd = xd2[j % 2]
                for ci, (r0, nr) in enumerate(D_CHUNKS):
                    ps = fps.tile([128, nr, 48], dt.float32, tag="fir", name=f"dp{j}_{ci}")
                    for t in PE_TAPS:
                        g = FIR_G[t]
                        nc.tensor.matmul(
                            ps[:],
                            itile[:, g * 128 : (g + 1) * 128],
                            raw[j + t][:, 3 + r0 : 3 + r0 + nr, 3:51],
                            start=(t == 0),
                            stop=(t == 4),
                        )
                    nc.vector.scalar_tensor_tensor(
                        xd[:, 3 + r0 : 3 + r0 + nr, 3:51],
                        raw[j + 2][:, 3 + r0 : 3 + r0 + nr, 3:51],
                        3.0,
                        ps[:],
                        mybir.AluOpType.mult,
                        mybir.AluOpType.add,
                    )

                # ---- h-pass (interior cols): x1[h, 3+c] = sum_t k_t xd[h+t, 3+c]
                x1 = x12[j % 2]
                for ci, (r0, nr) in enumerate(H_CHUNKS_49):
                    ps = fps.tile([128, nr, 48], dt.float32, tag="fir", name=f"hp{j}_{ci}")
                    for t in PE_TAPS:
                        g = FIR_G[t]
                        nc.tensor.matmul(
                            ps[:],
                            itile[:, g * 128 : (g + 1) * 128],
                            xd[:, r0 + t : r0 + t + nr, 3:51],
                            start=(t == 0),
                            stop=(t == 4),
                        )
                    nc.vector.scalar_tensor_tensor(
                        x1[:, r0 : r0 + nr, 3:51],
                        xd[:, r0 + 2 : r0 + 2 + nr, 3:51],
                        3.0,
                        ps[:],
                        mybir.AluOpType.mult,
                        mybir.AluOpType.add,
                    )

                # ---- w-pass: xf[h, w] = sum_t k_t x1[h, w+t], stored as
                # even/odd w-phases: xfj[:, ph, h, u] = xf[h, 2u+ph]
                xfj = xfpool.tile([128, 2, 49, 25], dt.float32r, tag="xf", name=f"xf{j}")
                for ci, (r0, nr) in enumerate(W_CHUNKS_49):
                    ps = fps.tile([128, nr, 50], dt.float32, tag="fir", name=f"wp{j}_{ci}")
                    for t in PE_TAPS:
                        g = FIR_G[t]
                        nc.tensor.matmul(
                            ps[:],
                            itile[:, g * 128 : (g + 1) * 128],
                            x1[:, r0 : r0 + nr, t : t + 50],
                            start=(t == 0),
                            stop=(t == 4),
                        )
                    nc.vector.scalar_tensor_tensor(
                        xfj[:, 0, r0 : r0 + nr, :],
                        x1[:, r0 : r0 + nr, 2:51:2],
                        3.0,
                        ps[:, :, 0:49:2],
                        mybir.AluOpType.mult,
                        mybir.AluOpType.add,
                    )
                    nc.vector.scalar_tensor_tensor(
                        xfj[:, 1, r0 : r0 + nr, :],
                        x1[:, r0 : r0 + nr, 3:52:2],
                        3.0,
                        ps[:, :, 1:50:2],
                        mybir.AluOpType.mult,
                        mybir.AluOpType.add,
                    )
                xf[j] = xfj

                # ---- conv for output plane sd once xf[2sd..2sd+2] ready
                if j >= 2 and j % 2 == 0:
                    sd = (j - 2) // 2
                    for b in range(2):
                        stage = stpool.tile(
                            [128, 24, 24], dt.float32, tag="st", name=f"st{sd}_{b}"
                        )
                        for hh in range(2):
                            pc = cps.tile(
                                [128, 12, 24], dt.float32, tag="conv", name=f"cv{sd}_{b}_{hh}"
                            )
                            tapi = 0
                            for kd in range(3):
                                src = xf[2 * sd + kd]
                                for kh in range(3):
                                    h0 = 24 * hh + kh
                                    for kw in range(3):
                                        ph, off = kw & 1, kw >> 1
                                        nc.tensor.matmul(
                                            pc[:],
                                            wtile[:, tapi * 256 + b * 128 : tapi * 256 + (b + 1) * 128],
                                            src[:, ph, h0 : h0 + 23 : 2, off : off + 24],
                                            start=(tapi == 0),
                                            stop=(tapi == 26),
                                        )
                                        tapi += 1
                            nc.scalar.activation(
                                stage[:, hh * 12 : (hh + 1) * 12, :],
                                pc[:],
                                mybir.ActivationFunctionType.Identity,
                                bias=btile[:, b : b + 1],
                            )
                        nc.sync.dma_start(o[b * 128 : (b + 1) * 128, sd, :, :], stage[:])

    nc.compile()
    return nc


def _prep_host_inputs(x, weight, bias):
    """Build per-core input maps."""
    x = np.ascontiguousarray(x, dtype=np.float32)
    w64 = np.asarray(weight, dtype=np.float64) / (11.0**3)
    # wt[ci, tap*256 + b*128 + oc] = w64[b*128+oc, ci, kd, kh, kw]
    wt = np.transpose(w64, (1, 2, 3, 4, 0)).reshape(128, 27 * 256)
    wt_np = _round_fp32r(wt.astype(np.float32))

    ident = np.zeros((128, 256), dtype=np.float32)
    ident[:, 0:128] = np.eye(128, dtype=np.float32)
    ident[:, 128:256] = 3.0 * np.eye(128, dtype=np.float32)

    biasb = np.ascontiguousarray(
        np.asarray(bias, dtype=np.float32).reshape(2, 128).T
    )  # [oc, b]

    # pad d,h by (3,2); pad w by (3,3) so plane width 54 keeps chunks even
    xp = np.pad(x, ((0, 0), (0, 0), (3, 2), (3, 2), (3, 3)))
    xp = _round_fp32r(xp)

    in_maps = []
    for core in range(N_CORES):
        n, dq = core // 4, core % 4
        slab = np.ascontiguousarray(xp[n, :, 12 * dq : 12 * dq + 17, :, :])
        in_maps.append({"xs": slab, "wt": wt_np, "ident": ident, "biasd": biasb})
    return in_maps


LAST_RESULTS = None


def kernel(x, weight, bias):
    global LAST_RESULTS
    _maybe_install_ntff_shim()

    from concourse.bass_utils import run_bass_kernel_spmd

    nc = _CACHE.get("nc")
    if nc is None:
        nc = _build_module()
        _CACHE["nc"] = nc

    in_maps = _prep_host_inputs(x, weight, bias)
    res = run_bass_kernel_spmd(nc, in_maps, core_ids=list(range(N_CORES)))
    LAST_RESULTS = res

    out = np.empty((2, 256, 24, 24, 24), dtype=np.float32)
    for core in range(N_CORES):
        n, dq = core // 4, core % 4
        out[n, :, 6 * dq : 6 * dq + 6, :, :] = res.results[core]["o"]
    return out


# revision 9
# speedup vs baseline: 1.6240x; 1.1181x over previous
"""Trainium2 Bass kernel for conv_downsample_3d (StyleGAN2-style):
separable 5-tap FIR smoothing ([1,3,3,3,1]/11 per axis) followed by a
3x3x3 stride-2 conv (128->256 ch) plus bias.

x: (2,128,48,48,48) f32, weight: (256,128,3,3,3), bias: (256,)
out: (2,256,24,24,24) f32

Strategy (8 NeuronCores, SPMD):
 - shard over (batch n in 2) x (output-depth quarter dq in 4); each core
   computes out[n, :, 6*dq:6*dq+6, :, :] (full 256 out-channels).
 - all matmuls in float32r (fp32 with 12-bit-rounded mantissa, full PE rate;
   moving sizes must be even and >=256, inner dims contiguous for pair rate):
   * FIR pass per axis = 5 PSUM-accumulated matmuls with k_t*I stationary
     and shifted rhs access patterns. Tap coefficients {1,3} and 12x12-bit
     products are exact; 1/11^3 normalization is folded into conv weights.
     d- and h- passes compute only the interior (borders stay exactly zero).
   * FIR output planes are stored split into even/odd w-phases so the
     stride-2 conv rhs reads contiguous spans (strided inner = half rate).
   * conv = 27 tap-matmuls accumulating into PSUM; bias fused into the
     ACT eviction.
"""

import numpy as np

_CACHE = {}

N_CORES = 8
FIR_G = [0, 1, 1, 1, 0]  # identity block per tap: 0 -> 1*I, 1 -> 3*I

# interior-only chunks: d-pass over 48 interior rows, inner 48
D_CHUNKS = [(0, 10), (10, 10), (20, 10), (30, 10), (40, 8)]  # rows of 48, N=480/384
H_CHUNKS_49 = [(0, 10), (10, 10), (20, 10), (30, 10), (40, 9)]  # rows of 49, N=480/432
W_CHUNKS_49 = [(0, 10), (10, 10), (20, 10), (30, 10), (40, 9)]  # rows of 49, N=500/450
PE_TAPS = [0, 1, 3, 4]  # center tap (t=2, coeff 3) fused into the DVE eviction


def _round_fp32r(a: np.ndarray) -> np.ndarray:
    """Round-to-nearest-even onto the fp32r grid (low 12 mantissa bits)."""
    a = np.ascontiguousarray(a, dtype=np.float32)
    u = a.view(np.uint32).astype(np.uint64)
    u = ((u + 0x7FF + ((u >> 12) & 1)) & ~np.uint64(0xFFF)).astype(np.uint32)
    return u.view(np.float32)


def _maybe_install_ntff_shim():
    """Best-effort: register the axon NTFF profiling hook so BASS_TRACE=1
    yields exec times. Harmless if unavailable."""
    try:
        import sys
        import types

        if "antenv.axon_hooks" not in sys.modules:
            mod = types.ModuleType("antenv.axon_hooks")
            holder = {"hook": None}
            mod.set_axon_ntff_profile_hook = lambda h: holder.__setitem__("hook", h)
            mod.get_axon_ntff_profile_hook = lambda: holder["hook"]
            sys.modules["antenv.axon_hooks"] = mod
        mod = sys.modules["antenv.axon_hooks"]
        if mod.get_axon_ntff_profile_hook() is None:
            from trn_agent_boot.trn_boot import _ntff_profile_via_ctypes

            mod.set_axon_ntff_profile_hook(
                _ntff_profile_via_ctypes("/opt/axon/libaxon_pjrt.so")
            )
    except Exception:
        pass


def _build_module():
    import concourse.bacc as bacc
    import concourse.mybir as mybir
    import concourse.tile as tile

    dt = mybir.dt
    nc = bacc.Bacc("TRN2", target_bir_lowering=False, debug=False)

    xs = nc.dram_tensor("xs", [128, 17, 53, 54], dt.float32r, kind="ExternalInput").ap()
    wt = nc.dram_tensor("wt", [128, 27 * 256], dt.float32r, kind="ExternalInput").ap()
    ident = nc.dram_tensor("ident", [128, 256], dt.float32r, kind="ExternalInput").ap()
    biasd = nc.dram_tensor("biasd", [128, 2], dt.float32, kind="ExternalInput").ap()
    o = nc.dram_tensor("o", [256, 6, 24, 24], dt.float32, kind="ExternalOutput").ap()

    with tile.TileContext(nc) as tc:
        with (
            tc.tile_pool(name="wp", bufs=1) as wpool,
            tc.tile_pool(name="raw", bufs=5) as rawpool,
            tc.tile_pool(name="midd", bufs=1) as midpool,
            tc.tile_pool(name="xfp", bufs=4) as xfpool,
            tc.tile_pool(name="stp", bufs=3) as stpool,
            tc.tile_pool(name="up", bufs=3) as upool,
            tc.tile_pool(name="fps", bufs=4, space="PSUM") as fps,
            tc.tile_pool(name="cps", bufs=4, space="PSUM") as cps,
        ):
            # identity first: needed by the very first matmul
            itile = wpool.tile([128, 256], dt.float32r, tag="ident", name="itile")
            nc.sync.dma_start(itile[:], ident[:])

            raw = {}

            def load_raw(p):
                t = rawpool.tile([128, 53, 54], dt.float32r, tag="raw", name=f"raw{p}")
                nc.sync.dma_start(t[:, 0:27, :], xs[:, p, 0:27, :])
                nc.sync.dma_start(t[:, 27:53, :], xs[:, p, 27:53, :])
                raw[p] = t

            for p in range(5):
                load_raw(p)

            # weights/bias: not needed until the first conv (j=2)
            wtile = wpool.tile([128, 27 * 256], dt.float32r, tag="wt", name="wtile")
            nc.sync.dma_start(wtile[:], wt[:])
            btile = wpool.tile([128, 2], dt.float32, tag="bias", name="btile")
            nc.sync.dma_start(btile[:], biasd[:])

            # persistent double-buffered intermediates with exactly-zero borders
            xd2 = [
                midpool.tile([128, 53, 54], dt.float32r, tag=f"xd{i}", name=f"xd{i}")
                for i in range(2)
            ]
            x12 = [
                midpool.tile([128, 49, 54], dt.float32r, tag=f"x1_{i}", name=f"x1_{i}")
                for i in range(2)
            ]
            for i in range(2):
                nc.gpsimd.memset(xd2[i][:].bitcast(dt.float32), 0.0)
                nc.gpsimd.memset(x12[i][:].bitcast(dt.float32), 0.0)

            xf = {}
            for j in range(13):
                if j + 5 <= 16:
                    load_raw(j + 5)

                # ---- d-pass (interior only): xd[3+r, 3+c] over 48x48
                xd = xd2[j % 2]
                for ci, (r0, nr) in enumerate(D_CHUNKS):
                    sl = (slice(None), slice(3 + r0, 3 + r0 + nr), slice(3, 51))
                    ud = upool.tile([128, nr, 48], dt.float32r, tag="ud", name=f"ud{j}_{ci}")
                    nc.vector.tensor_add(ud[:], raw[j + 1][sl], raw[j + 3][sl])
                    ps = fps.tile([128, nr, 48], dt.float32, tag="fir", name=f"dp{j}_{ci}")
                    nc.tensor.matmul(ps[:], itile[:, 0:128], raw[j][sl], start=True, stop=False)
                    nc.tensor.matmul(ps[:], itile[:, 128:256], ud[:], start=False, stop=False)
                    nc.tensor.matmul(ps[:], itile[:, 0:128], raw[j + 4][sl], start=False, stop=True)
                    nc.vector.scalar_tensor_tensor(
                        xd[sl],
                        raw[j + 2][sl],
                        3.0,
                        ps[:],
                        mybir.AluOpType.mult,
                        mybir.AluOpType.add,
                    )

                # ---- h-pass (interior cols): x1[h, 3+c] = sum_t k_t xd[h+t, 3+c]
                x1 = x12[j % 2]
                for ci, (r0, nr) in enumerate(H_CHUNKS_49):
                    uh = upool.tile([128, nr, 48], dt.float32r, tag="uh", name=f"uh{j}_{ci}")
                    nc.vector.tensor_add(
                        uh[:],
                        xd[:, r0 + 1 : r0 + 1 + nr, 3:51],
                        xd[:, r0 + 3 : r0 + 3 + nr, 3:51],
                    )
                    ps = fps.tile([128, nr, 48], dt.float32, tag="fir", name=f"hp{j}_{ci}")
                    nc.tensor.matmul(
                        ps[:], itile[:, 0:128], xd[:, r0 : r0 + nr, 3:51], start=True, stop=False
                    )
                    nc.tensor.matmul(ps[:], itile[:, 128:256], uh[:], start=False, stop=False)
                    nc.tensor.matmul(
                        ps[:], itile[:, 0:128], xd[:, r0 + 4 : r0 + 4 + nr, 3:51], start=False, stop=True
                    )
                    nc.vector.scalar_tensor_tensor(
                        x1[:, r0 : r0 + nr, 3:51],
                        xd[:, r0 + 2 : r0 + 2 + nr, 3:51],
                        3.0,
                        ps[:],
                        mybir.AluOpType.mult,
                        mybir.AluOpType.add,
                    )

                # ---- w-pass: xf[h, w] = sum_t k_t x1[h, w+t], stored as
                # even/odd w-phases: xfj[:, ph, h, u] = xf[h, 2u+ph]
                xfj = xfpool.tile([128, 2, 49, 25], dt.float32r, tag="xf", name=f"xf{j}")
                for ci, (r0, nr) in enumerate(W_CHUNKS_49):
                    ps = fps.tile([128, nr, 50], dt.float32, tag="fir", name=f"wp{j}_{ci}")
                    for t in PE_TAPS:
                        g = FIR_G[t]
                        nc.tensor.matmul(
                            ps[:],
                            itile[:, g * 128 : (g + 1) * 128],
                            x1[:, r0 : r0 + nr, t : t + 50],
                            start=(t == 0),
                            stop=(t == 4),
                        )
                    nc.vector.scalar_tensor_tensor(
                        xfj[:, 0, r0 : r0 + nr, :],
                        x1[:, r0 : r0 + nr, 2:51:2],
                        3.0,
                        ps[:, :, 0:49:2],
                        mybir.AluOpType.mult,
                        mybir.AluOpType.add,
                    )
                    nc.vector.scalar_tensor_tensor(
                        xfj[:, 1, r0 : r0 + nr, :],
                        x1[:, r0 : r0 + nr, 3:52:2],
                        3.0,
                        ps[:, :, 1:50:2],
                        mybir.AluOpType.mult,
                        mybir.AluOpType.add,
                    )
                xf[j] = xfj

                # ---- conv for output plane sd once xf[2sd..2sd+2] ready
                if j >= 2 and j % 2 == 0:
                    sd = (j - 2) // 2
                    for b in range(2):
                        stage = stpool.tile(
                            [128, 24, 24], dt.float32, tag="st", name=f"st{sd}_{b}"
                        )
                        for hh in range(2):
                            pc = cps.tile(
                                [128, 12, 24], dt.float32, tag="conv", name=f"cv{sd}_{b}_{hh}"
                            )
                            tapi = 0
                            for kd in range(3):
                                src = xf[2 * sd + kd]
                                for kh in range(3):
                                    h0 = 24 * hh + kh
                                    for kw in range(3):
                                        ph, off = kw & 1, kw >> 1
                                        nc.tensor.matmul(
                                            pc[:],
                                            wtile[:, tapi * 256 + b * 128 : tapi * 256 + (b + 1) * 128],
                                            src[:, ph, h0 : h0 + 23 : 2, off : off + 24],
                                            start=(tapi == 0),
                                            stop=(tapi == 26),
                                        )
                                        tapi += 1
                            nc.scalar.activation(
                                stage[:, hh * 12 : (hh + 1) * 12, :],
                                pc[:],
                                mybir.ActivationFunctionType.Identity,
                                bias=btile[:, b : b + 1],
                            )
                        nc.sync.dma_start(o[b * 128 : (b + 1) * 128, sd, :, :], stage[:])

    nc.compile()
    return nc


def _prep_host_inputs(x, weight, bias):
    """Build per-core input maps."""
    x = np.ascontiguousarray(x, dtype=np.float32)
    w64 = np.asarray(weight, dtype=np.float64) / (11.0**3)
    # wt[ci, tap*256 + b*128 + oc] = w64[b*128+oc, ci, kd, kh, kw]
    wt = np.transpose(w64, (1, 2, 3, 4, 0)).reshape(128, 27 * 256)
    wt_np = _round_fp32r(wt.astype(np.float32))

    ident = np.zeros((128, 256), dtype=np.float32)
    ident[:, 0:128] = np.eye(128, dtype=np.float32)
    ident[:, 128:256] = 3.0 * np.eye(128, dtype=np.float32)

    biasb = np.ascontiguousarray(
        np.asarray(bias, dtype=np.float32).reshape(2, 128).T
    )  # [oc, b]

    # pad d,h by (3,2); pad w by (3,3) so plane width 54 keeps chunks even
    xp = np.pad(x, ((0, 0), (0, 0), (3, 2), (3, 2), (3, 3)))
    xp = _round_fp32r(xp)

    in_maps = []
    for core in range(N_CORES):
        n, dq = core // 4, core % 4
        slab = np.ascontiguousarray(xp[n, :, 12 * dq : 12 * dq + 17, :, :])
        in_maps.append({"xs": slab, "wt": wt_np, "ident": ident, "biasd": biasb})
    return in_maps


LAST_RESULTS = None


def kernel(x, weight, bias):
    global LAST_RESULTS
    _maybe_install_ntff_shim()

    from concourse.bass_utils import run_bass_kernel_spmd

    nc = _CACHE.get("nc")
    if nc is None:
        nc = _build_module()
        _CACHE["nc"] = nc

    in_maps = _prep_host_inputs(x, weight, bias)
    res = run_bass_kernel_spmd(nc, in_maps, core_ids=list(range(N_CORES)))
    LAST_RESULTS = res

    out = np.empty((2, 256, 24, 24, 24), dtype=np.float32)
    for core in range(N_CORES):
        n, dq = core // 4, core % 4
        out[n, :, 6 * dq : 6 * dq + 6, :, :] = res.results[core]["o"]
    return out
